# revision 22
# baseline (speedup 1.0000x reference)
"""ChunkAwareAttention Trainium2 kernel (bf16 datapath), v2.

Model (hardcoded): B=4, T=2048, D=512, H=8, DK=64, CHUNK=64, EPS=1e-5.
  xn = LayerNorm(x) * ln_w + ln_b          (affine folded into W on host)
  q/k/v = heads(xn @ W{q,k,v} + b)         [B,H,T,DK]
  scores = (q @ (k + pk)^T) / sqrt(DK)     (pos term + bk folded into kT)
  chunk-causal mask, softmax, @v, out = concat_heads @ Wout + bout

Sharding over 8 cores: core c -> batch b = c//2, head-group g = c%2
(4 heads = 256 features per core). Host sums the two partials per batch.

v2 design (vs v1):
  - x shipped TRANSPOSED (tb-major [4][512d][512t]); LayerNorm stats via
    PE ones-matmuls producing partition-replicated sums, so no PE
    transposes and no bn_stats chain.  Normalize runs on GpSimd.
  - scores for the two heads of an m-group are row-packed: head A uses
    PE rows 0:63, head B rows 64:127 (tile_position auto-derived from
    base_partition) -> the two matmuls run concurrently.
  - score PSUM tile [128, 1024] = headA 512 q-cols | headB 512 q-cols
    (different banks); ONE exp per k-tile covers both heads.
  - chunk-diagonal masking via GpSimd memset-0 on the exp'd tile
    (replaces rank-1 -1e30 matmuls on PE).
  - softmax denominators via ones-columns in v (free: matmul cost is
    moving-column-bound); reciprocal reads PSUM directly.
  - wavefront schedule: attention q-blocks of 512 per m-group; LN
    normalize / projections / v / out-projection interleave into the
    ACT-bound attention stream to keep PE dense (HAM stays warm).
  - all sqrt before the first exp -> exactly 2 ACT table loads.
"""

import sys

if "/opt/trn_rl_repo" not in sys.path:
    sys.path.insert(0, "/opt/trn_rl_repo")

import math
import numpy as np
import ml_dtypes

import concourse.bass as bass
import concourse.tile as tile
from concourse import bacc, mybir
from concourse.bass_utils import run_bass_kernel_spmd

B, T, D, H = 4, 2048, 512, 8
DK = D // H
CHUNK = 64
EPS = 1e-5
NCORES = 8
HPC = H // 2          # heads per core = 4
F = HPC * DK          # features per core = 256
KD = D // 128         # contraction tiles over D = 4
NT = T // 128         # 128-row tiles over T = 16
F32 = mybir.dt.float32
BF16 = mybir.dt.bfloat16
SCALE = 1.0 / math.sqrt(DK)
LAG = 2
W = 512               # attention q-block width
NQB = T // W          # q-blocks = 4


def _build_program():
    nc = bacc.Bacc(
        "TRN2",
        target_bir_lowering=False,
        debug=False,
        enable_asserts=False,
        num_devices=NCORES,
    )

    # xt: tb-major transposed x: [4 tb][512 d][512 t] flattened.
    xt_d = nc.dram_tensor("xt", [T, D], BF16, kind="ExternalInput").ap()
    wq_d = nc.dram_tensor("wq", [D, F], BF16, kind="ExternalInput").ap()
    wk_d = nc.dram_tensor("wk", [D, F], BF16, kind="ExternalInput").ap()
    wv_d = nc.dram_tensor("wv", [D, F], BF16, kind="ExternalInput").ap()
    wout_d = nc.dram_tensor("wout", [F, D], BF16, kind="ExternalInput").ap()
    pkT_d = nc.dram_tensor("pkT", [F, T], BF16, kind="ExternalInput").ap()
    bq_d = nc.dram_tensor("bq", [F, 1], F32, kind="ExternalInput").ap()
    bv_d = nc.dram_tensor("bv", [1, F], F32, kind="ExternalInput").ap()
    out_d = nc.dram_tensor("out", [T, D], F32, kind="ExternalOutput").ap()

    with tile.TileContext(nc) as tc:
        _emit(nc, tc, xt_d, wq_d, wk_d, wv_d, wout_d, pkT_d,
              bq_d, bv_d, out_d)

    nc.compile()
    return nc


def _emit(nc, tc, xt_d, wq_d, wk_d, wv_d, wout_d, pkT_d, bq_d, bv_d, out_d):
    from contextlib import ExitStack

    ctx = ExitStack()
    with ctx:
        singles = ctx.enter_context(tc.tile_pool(name="singles", bufs=1))
        xsqpool = ctx.enter_context(tc.tile_pool(name="xsq", bufs=2))
        stpool = ctx.enter_context(tc.tile_pool(name="st", bufs=4))
        t1pool = ctx.enter_context(tc.tile_pool(name="t1", bufs=3))
        exppool = ctx.enter_context(tc.tile_pool(name="exp", bufs=4))
        rcpool = ctx.enter_context(tc.tile_pool(name="rc", bufs=4))
        opool = ctx.enter_context(tc.tile_pool(name="ostage", bufs=3))
        # PSUM: ps 2 x [128,1024] (4 banks) + oacc 2 x [128,512] (2) +
        # free 2 x [128,512] (2) = 8 banks.
        ps = ctx.enter_context(tc.tile_pool(name="ps", bufs=2, space="PSUM"))
        oaccp = ctx.enter_context(
            tc.tile_pool(name="oacc", bufs=2, space="PSUM"))
        freep = ctx.enter_context(
            tc.tile_pool(name="free", bufs=2, space="PSUM"))

        # ---- input DMAs (xt on sync queue, tb0 first; weights on gpsimd) --
        xtb = []
        for tb in range(NQB):
            t = singles.tile([128, KD * W], BF16, tag=f"xt{tb}",
                             name=f"xt{tb}")
            src = bass.AP(tensor=xt_d.tensor,
                          offset=xt_d.offset + tb * W * D,
                          ap=[[W, 128], [128 * W, KD], [1, W]])
            dst = t[:].rearrange("p (b c) -> p b c", c=W)
            q = nc.sync if tb % 2 == 0 else nc.gpsimd
            q.dma_start(out=dst, in_=src)
            xtb.append(t)

        def load_mono(dram, nblk, width, nm):
            t = singles.tile([128, nblk * width], BF16, tag=nm, name=nm)
            src = bass.AP(tensor=dram.tensor, offset=dram.offset,
                          ap=[[width, 128], [128 * width, nblk], [1, width]])
            dst = t[:].rearrange("p (b c) -> p b c", c=width)
            nc.gpsimd.dma_start(out=dst, in_=src)
            return t
        wq_t = load_mono(wq_d, KD, F, "wqall")
        wk_t = load_mono(wk_d, KD, F, "wkall")
        wv_t = load_mono(wv_d, KD, F, "wvall")
        wq_sb = [wq_t[:, kd * F:(kd + 1) * F] for kd in range(KD)]
        wk_sb = [wk_t[:, kd * F:(kd + 1) * F] for kd in range(KD)]
        wv_sb = [wv_t[:, kd * F:(kd + 1) * F] for kd in range(KD)]
        pkT_t = load_mono(pkT_d, 2, T, "pkTall")
        pkT_sb = [pkT_t[:, m * T:(m + 1) * T] for m in range(2)]
        wout_t = load_mono(wout_d, 2, D, "woutall")
        wout_sb = [wout_t[:, m * D:(m + 1) * D] for m in range(2)]
        bq_t = singles.tile([128, 2], F32, tag="bqall", name="bq_t")
        nc.sync.dma_start(out=bq_t[:], in_=bass.AP(
            tensor=bq_d.tensor, offset=bq_d.offset, ap=[[1, 128], [128, 2]]))
        bq_sb = [bq_t[:, m:m + 1] for m in range(2)]
        bv_sb = singles.tile([128, F], F32)
        nc.gpsimd.dma_start(
            out=bv_sb[:],
            in_=bass.AP(tensor=bv_d.tensor, offset=bv_d.offset,
                        ap=[[0, 128], [1, F]]))

        # ---- consts ----
        ones_t = singles.tile([128, 128], BF16, tag="ones")
        nc.vector.memset(ones_t[:], 1.0)
        eps_t = singles.tile([128, 1], F32)
        nc.vector.memset(eps_t, EPS)

        # ---- stats staging (partition-replicated) ----
        mu_all = singles.tile([128, T], BF16, tag="mu", name="mu_all")
        rstd_all = singles.tile([128, T], BF16, tag="rstd", name="rstd_all")
        murstd_all = singles.tile([128, T], BF16, tag="murstd",
                                  name="murstd_all")

        # ---- big resident activations (bf16) ----
        xnT = singles.tile([128, KD * T], BF16, tag="xnT", name="xnT")
        qT = [singles.tile([128, T], BF16, tag=f"qT{m}", name=f"qT{m}")
              for m in range(2)]
        kT = [singles.tile([128, T], BF16, tag=f"kT{m}", name=f"kT{m}")
              for m in range(2)]
        # V natural layout, per head [V_h(64) | ones(64)]: ones columns give
        # the softmax denominator on PSUM rows 64:128 of the attnv matmul.
        v_sb = [singles.tile([128, HPC * (2 * DK)], BF16, tag=f"v{ti}",
                             name=f"v{ti}") for ti in range(NT)]
        for ti in range(NT):
            nc.vector.memset(v_sb[ti][:], 1.0)  # pre-fill ones columns
        att = [singles.tile([128, T], BF16, tag=f"att{m}", name=f"att{m}")
               for m in range(2)]

        def xnT_c(kd, lo, hi):
            return xnT[:, kd * T + lo:kd * T + hi]

        # ====== LayerNorm stats for one 512-col t-block ======
        def emit_stats(tb):
            xsq = xsqpool.tile([128, KD * W], BF16)
            if tb == 0:
                nc.vector.tensor_tensor(
                    out=xsq[:], in0=xtb[tb][:], in1=xtb[tb][:],
                    op=mybir.AluOpType.mult)
            else:
                nc.scalar.square(out=xsq[:], in_=xtb[tb][:])
            psum = freep.tile([128, W], F32, tag="free", name="ps_sum")
            pssq = freep.tile([128, W], F32, tag="free", name="ps_sq")
            for kd in range(KD):
                nc.tensor.matmul(
                    psum[:], ones_t[:], xtb[tb][:, kd * W:(kd + 1) * W],
                    start=(kd == 0), stop=(kd == KD - 1))
            for kd in range(KD):
                nc.tensor.matmul(
                    pssq[:], ones_t[:], xsq[:, kd * W:(kd + 1) * W],
                    start=(kd == 0), stop=(kd == KD - 1))
            tsl = slice(tb * W, (tb + 1) * W)
            nc.vector.tensor_scalar_mul(
                out=mu_all[:, tsl], in0=psum[:], scalar1=1.0 / D)
            msq = stpool.tile([128, W], F32)
            nc.vector.tensor_scalar_mul(
                out=msq[:], in0=pssq[:], scalar1=1.0 / D)
            mumu = stpool.tile([128, W], F32)
            nc.vector.tensor_tensor(
                out=mumu[:], in0=mu_all[:, tsl], in1=mu_all[:, tsl],
                op=mybir.AluOpType.mult)
            var = stpool.tile([128, W], F32)
            nc.vector.tensor_tensor(
                out=var[:], in0=msq[:], in1=mumu[:],
                op=mybir.AluOpType.subtract)
            sd = stpool.tile([128, W], F32)
            nc.scalar.activation(
                out=sd[:], in_=var[:],
                func=mybir.ActivationFunctionType.Sqrt, bias=eps_t[:],
                scale=1.0)
            rstdf = stpool.tile([128, W], F32)
            nc.vector.reciprocal_approx_fast(out=rstdf[:], in_=sd[:])
            nc.vector.tensor_copy(out=rstd_all[:, tsl], in_=rstdf[:])
            nc.vector.tensor_tensor(
                out=murstd_all[:, tsl], in0=mu_all[:, tsl],
                in1=rstd_all[:, tsl], op=mybir.AluOpType.mult)

        # ====== normalize one (kd, tcn) tile (kd 0/1 DVE, 2/3 GpSimd) ======
        def emit_norm_x(tcn, kd):
            tsl = slice(tcn * W, (tcn + 1) * W)
            eng = nc.vector if kd < 2 else nc.gpsimd
            t1 = t1pool.tile([128, W], BF16)
            eng.tensor_tensor(
                out=t1[:], in0=xtb[tcn][:, kd * W:(kd + 1) * W],
                in1=rstd_all[:, tsl], op=mybir.AluOpType.mult)
            eng.tensor_tensor(
                out=xnT_c(kd, tcn * W, tcn * W + W), in0=t1[:],
                in1=murstd_all[:, tsl], op=mybir.AluOpType.subtract)

        # ====== q/k projections for (m, tcn) ======
        def emit_proj(m, tcn):
            tsl = slice(tcn * W, (tcn + 1) * W)
            msl = slice(m * 128, (m + 1) * 128)
            pq = freep.tile([128, W], F32, tag="free", name="pq")
            for kd in range(KD):
                nc.tensor.matmul(
                    pq[:], wq_sb[kd][:, msl], xnT_c(kd, tcn * W, tcn * W + W),
                    start=(kd == 0), stop=(kd == KD - 1))
            nc.vector.tensor_scalar_add(
                out=qT[m][:, tsl], in0=pq[:], scalar1=bq_sb[m])
            pk = freep.tile([128, W], F32, tag="free", name="pk")
            for kd in range(KD):
                nc.tensor.matmul(
                    pk[:], wk_sb[kd][:, msl], xnT_c(kd, tcn * W, tcn * W + W),
                    start=(kd == 0), stop=(kd == KD - 1))
            # kT = pk + pkT  (host pos projection, bk folded in)
            nc.vector.tensor_tensor(
                out=kT[m][:, tsl], in0=pk[:], in1=pkT_sb[m][:, tsl],
                op=mybir.AluOpType.add)

        # ====== v projection for one 128-row tile ======
        def emit_v(ti):
            pv = freep.tile([128, F], F32, tag="free", name="pv")
            for kd in range(KD):
                nc.tensor.matmul(
                    pv[:], xnT_c(kd, ti * 128, ti * 128 + 128), wv_sb[kd],
                    start=(kd == 0), stop=(kd == KD - 1))
            vt = v_sb[ti]
            dst = vt[:].rearrange("p (h c) -> p h c", h=HPC)[:, :, 0:DK]
            srcv = pv[:].rearrange("p (h c) -> p h c", c=DK)
            bvb = bv_sb[:].rearrange("p (h c) -> p h c", c=DK)
            nc.vector.tensor_tensor(
                out=dst, in0=srcv, in1=bvb, op=mybir.AluOpType.add)

        # ====== attention: one global pipeline over (qb, m, ki) units ======
        # scores+exp run LAG units ahead of attnv; block N+1's scores
        # overlap block N's attnv tail + softmax norms (no flush stall).
        def emit_scores(m, qb, ki):
            g = qb * W
            qcs = 0 if ki < 4 * qb else 128 * (ki - 4 * qb)
            spt = ps.tile([128, 1024], F32, tag="ps", name="spt")
            for hh in range(2):
                r0 = DK * hh
                nc.tensor.matmul(
                    spt[:, W * hh + qcs:W * hh + W],
                    kT[m][r0:r0 + DK, 128 * ki:128 * ki + 128],
                    qT[m][r0:r0 + DK, g + qcs:g + W],
                    start=True, stop=True)
            et = exppool.tile([128, 1024], BF16, tag="et", name="et")
            if qcs:
                esrc = spt[:].rearrange("p (b c) -> p b c", c=W)[:, :, qcs:W]
                edst = et[:].rearrange("p (b c) -> p b c", c=W)[:, :, qcs:W]
            else:
                esrc, edst = spt[:], et[:]
            nc.scalar.activation(
                out=edst, in_=esrc,
                func=mybir.ActivationFunctionType.Exp, scale=SCALE)
            if ki >= 4 * qb:
                # mask: key-chunk 2ki+1 (rows 64:) vs query-chunk 2ki
                for hh in range(2):
                    nc.gpsimd.memset(
                        et[DK:128, W * hh + qcs:W * hh + qcs + CHUNK], 0.0)
            return et

        def emit_attention(inject_map):
            oaccs = {}
            pend = []

            def do_attnv(m, qb, ki, et):
                kn = 4 * qb + 4
                if ki == 0:
                    # (m0, qb3) uses the free pool (idle after last proj) to
                    # break the oacc rotation stall at the qb2->qb3 boundary
                    pool = freep if (m, qb) == (0, 3) else oaccp
                    oaccs[(m, qb)] = [
                        pool.tile([128, W], F32,
                                  tag="free" if pool is freep else "oacc",
                                  name=f"oa{m}{qb}{hh}") for hh in range(2)]
                oacc = oaccs[(m, qb)]
                qcs = 0 if ki < 4 * qb else 128 * (ki - 4 * qb)
                for hh in range(2):
                    vst = v_sb[ki][:, (2 * m + hh) * 2 * DK:
                                   (2 * m + hh + 1) * 2 * DK]
                    nc.tensor.matmul(
                        oacc[hh][:, qcs:W],
                        vst, et[:, W * hh + qcs:W * hh + W],
                        start=(ki == 0), stop=(ki == kn - 1))
                if ki == kn - 1:
                    # softmax norms (denominator copy to SBUF first —
                    # reciprocal_approx_fast misreads PSUM on HW)
                    for hh in range(2):
                        dn = rcpool.tile([DK, W], F32, tag="dn", name="dn")
                        nc.vector.tensor_copy(
                            out=dn[:], in_=oacc[hh][DK:2 * DK, :])
                        rc = rcpool.tile([DK, W], F32, tag="rc", name="rc")
                        nc.vector.reciprocal_approx_fast(out=rc[:], in_=dn[:])
                        nc.vector.tensor_tensor(
                            out=att[m][DK * hh:DK * hh + DK,
                                       qb * W:qb * W + W],
                            in0=oacc[hh][0:DK, :], in1=rc[:],
                            op=mybir.AluOpType.mult)
                    del oaccs[(m, qb)]

            idx = 0
            for qb in range(NQB):
                for m in range(2):
                    for ki in range(4 * qb + 4):
                        et = emit_scores(m, qb, ki)
                        pend.append((m, qb, ki, et))
                        if len(pend) > LAG:
                            do_attnv(*pend.pop(0))
                        for piece in inject_map.get(idx, []):
                            piece()
                        idx += 1
            for u in pend:
                do_attnv(*u)

        # ====== output projection for q-block ======
        def emit_outproj(qb, tis=None):
            for ti in (tis if tis is not None else range(qb * 4, qb * 4 + 4)):
                po = ps.tile([128, W], F32, tag="ps", name="po")
                for m2 in range(2):
                    nc.tensor.matmul(
                        po[:], att[m2][:, ti * 128:(ti + 1) * 128],
                        wout_sb[m2], start=(m2 == 0), stop=(m2 == 1))
                og = opool.tile([128, W], F32, tag="og", name="og")
                nc.vector.tensor_copy(out=og[:], in_=po[:])
                nc.sync.dma_start(
                    out=out_d[ti * 128:(ti + 1) * 128, :], in_=og[:])

        # ====== emission schedule ======
        # units: qb0 -> idx 0-7, qb1 -> 8-23, qb2 -> 24-47, qb3 -> 48-79
        def prep_pieces(tcn):
            t0 = 4 * tcn
            return [
                lambda: (emit_norm_x(tcn, 0), emit_norm_x(tcn, 1)),
                lambda: emit_norm_x(tcn, 2),
                lambda: emit_norm_x(tcn, 3),
                lambda: emit_proj(0, tcn),
                lambda: emit_proj(1, tcn),
                lambda: emit_v(t0),
                lambda: emit_v(t0 + 1),
                lambda: (emit_v(t0 + 2), emit_v(t0 + 3)),
            ]

        def op_pieces(qb):
            return [lambda ti=ti: emit_outproj(qb, tis=[ti])
                    for ti in range(qb * 4, qb * 4 + 4)]

        for tb in range(NQB):
            emit_stats(tb)
        for piece in prep_pieces(0):
            piece()

        inject_map = {}
        for i, piece in enumerate(prep_pieces(1)):
            inject_map.setdefault(0 + i, []).append(piece)     # during qb0
        for i, piece in enumerate(prep_pieces(2)):
            inject_map.setdefault(8 + i, []).append(piece)     # during qb1 m0
        for i, piece in enumerate(op_pieces(0)):
            inject_map.setdefault(18 + i, []).append(piece)    # during qb1 m1
        for i, piece in enumerate(prep_pieces(3)):
            inject_map.setdefault(24 + i, []).append(piece)    # during qb2 m0
        for i, piece in enumerate(op_pieces(1)):
            inject_map.setdefault(38 + i, []).append(piece)    # during qb2 m1
        for i, piece in enumerate(op_pieces(2)):
            inject_map.setdefault(52 + i, []).append(piece)    # during qb3
        emit_attention(inject_map)
        emit_outproj(3)


_CACHED_NC = None


def _get_nc():
    global _CACHED_NC
    if _CACHED_NC is None:
        _CACHED_NC = _build_program()
    return _CACHED_NC


def make_in_maps(x, pos_enc, mask, ln_w, ln_b, Wq, bq, Wk, bk, Wv, bv,
                 Wpos, Wout, bout):
    f32, bf = np.float32, ml_dtypes.bfloat16
    x = np.asarray(x, f32)
    pos_enc = np.asarray(pos_enc, f32)
    ln_w = np.asarray(ln_w, f32)
    ln_b = np.asarray(ln_b, f32)
    Wq, bq = np.asarray(Wq, f32), np.asarray(bq, f32)
    Wk, bk = np.asarray(Wk, f32), np.asarray(bk, f32)
    Wv, bv = np.asarray(Wv, f32), np.asarray(bv, f32)
    Wpos = np.asarray(Wpos, f32)
    Wout = np.asarray(Wout, f32)

    # Fold the LayerNorm affine into the projections (exact rewrite).
    lw = ln_w[:, None]
    Wq_f, bq_f = Wq * lw, bq + ln_b @ Wq
    Wk_f, bk_f = Wk * lw, bk + ln_b @ Wk
    Wv_f, bv_f = Wv * lw, bv + ln_b @ Wv

    # Host-side pos projection (+ bk), shipped transposed.
    pk_full = pos_enc[0] @ Wpos  # [T, D]

    in_maps = []
    for c in range(NCORES):
        b, g = divmod(c, 2)
        hs = slice(g * F, (g + 1) * F)
        # x transposed, tb-major: [4 tb][512 d][512 t]
        xt = np.ascontiguousarray(
            x[b].T.reshape(D, NQB, W).transpose(1, 0, 2).reshape(T, D))
        in_maps.append({
            "xt": xt.astype(bf),
            "wq": np.ascontiguousarray(Wq_f[:, hs]).astype(bf),
            "wk": np.ascontiguousarray(Wk_f[:, hs]).astype(bf),
            "wv": np.ascontiguousarray(Wv_f[:, hs]).astype(bf),
            "wout": np.ascontiguousarray(Wout[hs, :]).astype(bf),
            "pkT": np.ascontiguousarray(
                (pk_full[:, hs] + bk_f[hs]).T).astype(bf),
            "bq": np.ascontiguousarray(bq_f[hs, None]),
            "bv": np.ascontiguousarray(bv_f[None, hs]),
        })
    return in_maps


def kernel(**inputs):
    in_maps = make_in_maps(**inputs)
    bout = np.asarray(inputs["bout"], np.float32)
    nc = _get_nc()
    res = run_bass_kernel_spmd(nc, in_maps, core_ids=list(range(NCORES)))

    out = np.empty((B, T, D), np.float32)
    for b in range(B):
        out[b] = res.results[2 * b]["out"] + res.results[2 * b + 1]["out"] + bout
    return out
